# revision 8
# baseline (speedup 1.0000x reference)
"""KoLeo loss kernel for Trainium2 (8 NeuronCores, SPMD).

Strategy:
  - Shard rows of student_output [8192, 768] across 8 cores (1024 rows each).
  - Each core computes min_j (||x_i||^2 + ||x_j||^2 - 2 x_i . x_j) for its
    rows i against ALL rows j via bf16 Gram-matrix tiles on the PE array.
  - The sq_j row vector (plus a +BIG diagonal mask) is folded into a fused
    DVE tensor_tensor_reduce (add + row-min), so the inner loop is just
    6 matmuls + 1 DVE op per [128, 512] tile.
  - Per-core inputs are column-ROTATED by the core's row offset so the same
    SPMD program works on every core (diagonal always lands in n-blocks 0/1).
  - Host does the cheap O(n) tail: d2 = min + sq_i, -mean(log(sqrt(d2)+eps)).
"""

import os

import numpy as np

try:
    import concourse  # noqa: F401
except ImportError:  # pragma: no cover - harness env fallback
    import sys

    sys.path.insert(0, "/opt/trn_rl_repo")

import concourse.bacc as bacc
import concourse.tile as tile
from concourse import mybir
from concourse.bass_utils import run_bass_kernel_spmd

N = 8192
D = 768
NCORES = 8
ROWS_PER_CORE = N // NCORES  # 1024
KCH = D // 128  # 6 k-chunks
MT = ROWS_PER_CORE // 128  # 8 m-tiles per core
NT = N // 512  # 16 n-tiles
QB = 4  # column blocks of 2048 (4 n-tiles each)
BIG = 1.0e30
EPS = 1e-8

TRACE = os.environ.get("KOLEO_TRACE", "0") == "1"
LAST = None  # BassKernelResults stash for test harness

_NC = None


def _build_nc(reps: int = 1):
    f32 = mybir.dt.float32
    bf16 = mybir.dt.bfloat16

    nc = bacc.Bacc("TRN2", target_bir_lowering=False, debug=False, num_devices=NCORES)

    xt_d = nc.declare_dram_parameter("xt", [KCH, 128, N], bf16, isOutput=False)
    xts_d = nc.declare_dram_parameter(
        "xts", [KCH, 128, ROWS_PER_CORE], bf16, isOutput=False
    )
    sqrep_d = nc.declare_dram_parameter("sqrep", [QB, 128, 2048], f32, isOutput=False)
    sqpatch_d = nc.declare_dram_parameter(
        "sqpatch", [MT, 128, 512], f32, isOutput=False
    )
    minred_d = nc.declare_dram_parameter("minred", [128, MT], f32, isOutput=True)

    with tile.TileContext(nc) as tc:
        with (
            tc.tile_pool(name="const", bufs=1) as cpool,
            tc.tile_pool(name="psum", bufs=8, space="PSUM") as psum_pool,
            tc.tile_pool(name="scratch", bufs=4) as spool,
        ):
            # --- persistent SBUF tiles ---
            xts_t = []
            for k in range(KCH):
                t = cpool.tile([128, ROWS_PER_CORE], bf16, tag=f"xts{k}")
                nc.sync.dma_start(t[:], xts_d[k])
                xts_t.append(t)

            sqpatch_t = []
            for mi in range(MT):
                t = cpool.tile([128, 512], f32, tag=f"sqp{mi}")
                nc.sync.dma_start(t[:], sqpatch_d[mi])
                sqpatch_t.append(t)

            sqrep_t = []
            for q in range(QB):
                t = cpool.tile([128, 2048], f32, tag=f"sqr{q}")
                nc.sync.dma_start(t[:], sqrep_d[q])
                sqrep_t.append(t)

            # xt loaded as (q, k) blocks of [128, 2048] so compute can start
            # after the first column block lands.
            xt_t = {}
            for q in range(QB):
                for k in range(KCH):
                    t = cpool.tile([128, 2048], bf16, tag=f"xt{q}_{k}")
                    nc.sync.dma_start(t[:], xt_d[k, :, q * 2048 : (q + 1) * 2048])
                    xt_t[(q, k)] = t

            minbuf = cpool.tile([128, MT, NT], f32, tag="minbuf")
            minred_t = cpool.tile([128, MT], f32, tag="minred")

            # --- main compute ---
            def body(_i=None):
                for q in range(QB):
                    for mi in range(MT):
                        for ni in range(4):
                            ng = q * 4 + ni
                            ps = psum_pool.tile([128, 512], f32, tag="ps")
                            for k in range(KCH):
                                nc.tensor.matmul(
                                    ps[:],
                                    xts_t[k][:, mi * 128 : (mi + 1) * 128],
                                    xt_t[(q, k)][:, ni * 512 : (ni + 1) * 512],
                                    start=(k == 0),
                                    stop=(k == KCH - 1),
                                )
                            if ng == mi // 4:
                                in1 = sqpatch_t[mi][:]
                            else:
                                in1 = sqrep_t[q][:, ni * 512 : (ni + 1) * 512]
                            sc = spool.tile([128, 512], bf16, tag="sc")
                            nc.vector.tensor_tensor(
                                sc[:], ps[:], in1, op=mybir.AluOpType.add
                            )
                            nc.vector.tensor_reduce(
                                minbuf[:, mi, ng : ng + 1],
                                sc[:],
                                axis=mybir.AxisListType.X,
                                op=mybir.AluOpType.min,
                            )

                for mi in range(MT):
                    nc.vector.tensor_reduce(
                        minred_t[:, mi : mi + 1],
                        minbuf[:, mi, :],
                        axis=mybir.AxisListType.X,
                        op=mybir.AluOpType.min,
                    )

            if reps == 1:
                body()
            else:
                with tc.For_i(0, reps, 1) as _i:
                    body(_i)

            nc.sync.dma_start(minred_d[:], minred_t[:])

    nc.compile()
    return nc


def _make_in_maps(x: np.ndarray):
    import ml_dtypes

    sq = np.einsum("nd,nd->n", x, x).astype(np.float32)  # [N]
    xt_bf = np.ascontiguousarray(x.T).astype(ml_dtypes.bfloat16)  # [D, N]

    in_maps = []
    for c in range(NCORES):
        shift = c * ROWS_PER_CORE
        xt_rot = np.ascontiguousarray(np.roll(xt_bf, -shift, axis=1)).reshape(
            KCH, 128, N
        )
        sq_rot = np.roll(sq, -shift)
        # sqrep as [QB, 128, 2048]: block q covers columns q*2048..(q+1)*2048
        sqrep = np.empty((QB, 128, 2048), np.float32)
        for q in range(QB):
            sqrep[q] = np.broadcast_to(sq_rot[q * 2048 : (q + 1) * 2048], (128, 2048))
        sqpatch = np.empty((MT, 128, 512), np.float32)
        for mi in range(MT):
            nb = mi // 4
            off = (mi % 4) * 128
            pat = np.broadcast_to(
                sq_rot[nb * 512 : (nb + 1) * 512], (128, 512)
            ).copy()
            pat[np.arange(128), off + np.arange(128)] += BIG
            sqpatch[mi] = pat
        xts = np.ascontiguousarray(
            (-2.0 * x[shift : shift + ROWS_PER_CORE].T)
        ).astype(ml_dtypes.bfloat16).reshape(KCH, 128, ROWS_PER_CORE)
        in_maps.append(
            {"xt": xt_rot, "xts": xts, "sqrep": sqrep, "sqpatch": sqpatch}
        )
    return in_maps, sq


def kernel(student_output: np.ndarray) -> np.ndarray:
    global _NC, LAST

    x = np.asarray(student_output, dtype=np.float32)
    assert x.shape == (N, D)
    in_maps, sq = _make_in_maps(x)

    if _NC is None:
        _NC = _build_nc()

    res = run_bass_kernel_spmd(_NC, in_maps, list(range(NCORES)), trace=TRACE)
    LAST = res
    results = res.results

    mins = np.concatenate(
        [np.asarray(results[c]["minred"]).T.reshape(-1) for c in range(NCORES)]
    )  # [N] ordered by global row
    d2 = np.maximum(mins.astype(np.float64) + sq.astype(np.float64), 0.0)
    val = -np.mean(np.log(np.sqrt(d2) + EPS))
    return np.array(val, dtype=np.float32)
